# revision 3
# baseline (speedup 1.0000x reference)
"""Causal grouped Conv1d on 8 Trainium2 NeuronCores.

Problem: x [B=4, L=4096, D=2048] f32, w [K=4, D/G=256, D=2048] f32, G=8 groups.
out[b, l, o] = sum_{k, i} x[b, l-3+k, g(o)*256 + i] * w[k, i, o]   (causal pad 3)

Sharding: data-parallel over (B, L/2) -> 8 shards of 2048 tokens each.
Each core gets its token rows plus a 3-row left halo (zeros at batch start).

Per-core kernel (Bass/Tile):
  - PE-transpose x tiles [128 tok, 128 cin] -> xT [128 cin, 131 tok] (incl halo)
  - grouped matmuls: psum[128 tok, 256 och] += xT[:, k:k+128].T @ w[k, cin, och]
    accumulating over k in 0..3 and the group's two 128-cin chunks.
"""

import numpy as np

import concourse.bass as bass
import concourse.mybir as mybir
import concourse.tile as tile
from concourse import bacc, masks
from concourse.bass_utils import run_bass_kernel_spmd

B, L, D, K, G = 4, 4096, 2048, 4, 8
CG = D // G          # 256 channels per group
NCORES = 8
TOK = (B * L) // NCORES   # 2048 tokens per core
TT = 128                  # token tile
NT = TOK // TT            # 16 token tiles
NCHUNK = D // 128         # 16 cin chunks of 128
PAD = K - 1               # 3 (causal left pad)

DT = mybir.dt.float32     # matmul input dtype knob (float32 | float32r)
F32 = mybir.dt.float32


def _emit(tc, nc, xs, wt, y):
    import contextlib
    ctx = contextlib.ExitStack()
    with ctx:
        constp = ctx.enter_context(tc.tile_pool(name="constp", bufs=1))
        wp = ctx.enter_context(tc.tile_pool(name="wp", bufs=1))
        xinp = ctx.enter_context(tc.tile_pool(name="xinp", bufs=3))
        halop = ctx.enter_context(tc.tile_pool(name="halop", bufs=3))
        xtp = ctx.enter_context(tc.tile_pool(name="xtp", bufs=20))
        outp = ctx.enter_context(tc.tile_pool(name="outp", bufs=3))
        ptr = ctx.enter_context(tc.tile_pool(name="ptr", bufs=3, space="PSUM"))
        po = ctx.enter_context(tc.tile_pool(name="po", bufs=4, space="PSUM"))

        ident = constp.tile([128, 128], DT)
        masks.make_identity(nc, ident[:])

        # Weights resident in SBUF: w_sb[(k, j)] = [128 cin, 2048 och]
        wsb = {}
        for k in range(K):
            for j in range(2):
                w_t = wp.tile([128, D], DT, name=f"w_{k}_{j}")
                nc.sync.dma_start(w_t[:], wt[k, j * 128:(j + 1) * 128, :])
                wsb[(k, j)] = w_t

        for t in range(NT):
            xm = xinp.tile([TT, D], DT, name="xm")
            nc.sync.dma_start(xm[:], xs[PAD + t * TT: PAD + (t + 1) * TT, :])
            xh = halop.tile([PAD, D], DT, name="xh")
            nc.sync.dma_start(xh[:], xs[t * TT: t * TT + PAD, :])

            # Transpose all 16 cin chunks for this token tile.
            xts = []
            for c in range(NCHUNK):
                ptile = ptr.tile([128, TT + PAD], DT, name="ptile")
                nc.tensor.transpose(
                    ptile[:, PAD:TT + PAD], xm[:, c * 128:(c + 1) * 128], ident[:]
                )
                nc.tensor.transpose(
                    ptile[:, 0:PAD], xh[:, c * 128:(c + 1) * 128], ident[0:PAD, 0:PAD]
                )
                xt_t = xtp.tile([128, TT + PAD], DT, name="xt_t")
                nc.vector.tensor_copy(xt_t[:], ptile[:])
                xts.append(xt_t)

            ot = outp.tile([TT, D], F32, name="ot")
            for g in range(G):
                pot = po.tile([TT, CG], F32, name="pot")
                first = True
                for j in range(2):
                    xt_t = xts[2 * g + j]
                    for k in range(K):
                        nc.tensor.matmul(
                            pot[:],
                            xt_t[:, k:k + TT],
                            wsb[(k, j)][:, g * CG:(g + 1) * CG],
                            start=first,
                            stop=(j == 1 and k == K - 1),
                        )
                        first = False
                nc.vector.tensor_copy(ot[:, g * CG:(g + 1) * CG], pot[:])
            nc.sync.dma_start(y[t * TT:(t + 1) * TT, :], ot[:])


_NC_CACHE = None


def build_nc():
    global _NC_CACHE
    if _NC_CACHE is not None:
        return _NC_CACHE
    nc = bacc.Bacc(
        "TRN2", target_bir_lowering=False, debug=False, num_devices=NCORES
    )
    xs = nc.dram_tensor("xs", [TOK + PAD, D], DT, kind="ExternalInput").ap()
    wt = nc.dram_tensor("wt", [K, CG, D], DT, kind="ExternalInput").ap()
    y = nc.dram_tensor("y", [TOK, D], F32, kind="ExternalOutput").ap()
    with tile.TileContext(nc) as tc:
        _emit(tc, nc, xs, wt, y)
    nc.compile()
    _NC_CACHE = nc
    return nc


def make_in_maps(x, w):
    """Shard full x [B, L, D] into 8 per-core inputs with causal halo rows."""
    x = np.ascontiguousarray(x, dtype=np.float32)
    w = np.ascontiguousarray(w, dtype=np.float32)
    halves = L // (NCORES // B)  # 2048
    in_maps = []
    for core in range(NCORES):
        b, h = divmod(core, NCORES // B)
        lo = h * halves
        shard = np.zeros((TOK + PAD, D), dtype=np.float32)
        if lo == 0:
            shard[PAD:] = x[b, lo:lo + TOK]
        else:
            shard[:] = x[b, lo - PAD:lo + TOK]
        in_maps.append({"xs": shard, "wt": w})
    return in_maps


def run(x, w, trace=False, **kw):
    nc = build_nc()
    res = run_bass_kernel_spmd(
        nc, make_in_maps(x, w), core_ids=list(range(NCORES)), trace=trace, **kw
    )
    halves = L // (NCORES // B)
    out = np.empty((B, L, D), dtype=np.float32)
    for core in range(NCORES):
        b, h = divmod(core, NCORES // B)
        out[b, h * halves:(h + 1) * halves] = res.results[core]["y"]
    return out, res


def kernel(x, w):
    out, _ = run(x, w, trace=False)
    return out


# revision 10
# speedup vs baseline: 2.1303x; 2.1303x over previous
"""Causal grouped Conv1d on 8 Trainium2 NeuronCores.

Problem: x [B=4, L=4096, D=2048] f32, w [K=4, D/G=256, D=2048] f32, G=8 groups.
out[b, l, o] = sum_{k, i} x[b, l-3+k, g(o)*256 + i] * w[k, i, o]   (causal pad 3)

Sharding: data-parallel over (B, L/2) -> 8 shards of 2048 tokens each.
Each core gets its token rows plus a 3-row left halo (zeros at batch start).

Per-core kernel (Bass/Tile):
  - PE-transpose x tiles [128 tok, 128 cin] -> xT [128 cin, 131 tok] (incl halo)
  - grouped matmuls: psum[128 tok, 256 och] += xT[:, k:k+128].T @ w[k, cin, och]
    accumulating over k in 0..3 and the group's two 128-cin chunks.
"""

import numpy as np

import concourse.bass as bass
import concourse.mybir as mybir
import concourse.tile as tile
from concourse import bacc, masks
from concourse.bass_utils import run_bass_kernel_spmd

B, L, D, K, G = 4, 4096, 2048, 4, 8
CG = D // G          # 256 channels per group
NCORES = 8
TOK = (B * L) // NCORES   # 2048 tokens per core
TT = 128                  # token tile
NT = TOK // TT            # 16 token tiles
NCHUNK = D // 128         # 16 cin chunks of 128
PAD = K - 1               # 3 (causal left pad)

DT = mybir.dt.float32     # storage/transpose dtype
F32 = mybir.dt.float32
F32R = mybir.dt.float32r
MM_F32R = True            # run conv matmuls in float32r


def _emit(tc, nc, xs, wt, y):
    import contextlib
    ctx = contextlib.ExitStack()
    with ctx:
        constp = ctx.enter_context(tc.tile_pool(name="constp", bufs=1))
        wp = ctx.enter_context(tc.tile_pool(name="wp", bufs=1))
        xinp = ctx.enter_context(tc.tile_pool(name="xinp", bufs=3))
        halop = ctx.enter_context(tc.tile_pool(name="halop", bufs=3))
        xtp = ctx.enter_context(tc.tile_pool(name="xtp", bufs=20))
        outp = ctx.enter_context(tc.tile_pool(name="outp", bufs=3))
        ptr = ctx.enter_context(tc.tile_pool(name="ptr", bufs=3, space="PSUM"))
        po = ctx.enter_context(tc.tile_pool(name="po", bufs=4, space="PSUM"))

        ident = constp.tile([128, 128], DT)
        masks.make_identity(nc, ident[:])

        # Weights resident in SBUF: w_sb[(k, j)] = [128 cin, 2048 och]
        wdt = F32R if MM_F32R else DT
        wsb = {}
        for k in range(K):
            for j in range(2):
                w_t = wp.tile([128, D], wdt, name=f"w_{k}_{j}")
                if MM_F32R:
                    # SWDGE cast fp32 -> fp32r (rounds to fp32r grid)
                    nc.gpsimd.dma_start(w_t[:], wt[k, j * 128:(j + 1) * 128, :])
                else:
                    nc.sync.dma_start(w_t[:], wt[k, j * 128:(j + 1) * 128, :])
                wsb[(k, j)] = w_t

        for t in range(NT):
            xm = xinp.tile([TT, D], DT, name="xm")
            nc.sync.dma_start(xm[:], xs[PAD + t * TT: PAD + (t + 1) * TT, :])
            xh = halop.tile([PAD, D], DT, name="xh")
            nc.sync.dma_start(xh[:], xs[t * TT: t * TT + PAD, :])

            # Transpose all 16 cin chunks for this token tile.
            xts = []
            for c in range(NCHUNK):
                ptile = ptr.tile([128, TT + PAD], DT, name="ptile")
                nc.tensor.transpose(
                    ptile[:, PAD:TT + PAD], xm[:, c * 128:(c + 1) * 128], ident[:]
                )
                nc.tensor.transpose(
                    ptile[:, 0:PAD], xh[:, c * 128:(c + 1) * 128], ident[0:PAD, 0:PAD]
                )
                xt_t = xtp.tile(
                    [128, TT + PAD], F32R if MM_F32R else DT, name="xt_t"
                )
                nc.vector.tensor_copy(xt_t[:], ptile[:])
                xts.append(xt_t)

            ot = outp.tile([TT, D], F32, name="ot")
            for g in range(G):
                pot = po.tile([TT, CG], F32, name="pot")
                first = True
                for j in range(2):
                    xt_t = xts[2 * g + j]
                    for k in range(K):
                        nc.tensor.matmul(
                            pot[:],
                            xt_t[:, k:k + TT],
                            wsb[(k, j)][:, g * CG:(g + 1) * CG],
                            start=first,
                            stop=(j == 1 and k == K - 1),
                        )
                        first = False
                nc.vector.tensor_copy(ot[:, g * CG:(g + 1) * CG], pot[:])
            nc.sync.dma_start(y[t * TT:(t + 1) * TT, :], ot[:])


_NC_CACHE = None


def build_nc():
    global _NC_CACHE
    if _NC_CACHE is not None:
        return _NC_CACHE
    nc = bacc.Bacc(
        "TRN2", target_bir_lowering=False, debug=False, num_devices=NCORES
    )
    xs = nc.dram_tensor("xs", [TOK + PAD, D], DT, kind="ExternalInput").ap()
    wt = nc.dram_tensor("wt", [K, CG, D], DT, kind="ExternalInput").ap()
    y = nc.dram_tensor("y", [TOK, D], F32, kind="ExternalOutput").ap()
    with tile.TileContext(nc) as tc:
        _emit(tc, nc, xs, wt, y)
    nc.compile()
    _NC_CACHE = nc
    return nc


def make_in_maps(x, w):
    """Shard full x [B, L, D] into 8 per-core inputs with causal halo rows."""
    x = np.ascontiguousarray(x, dtype=np.float32)
    w = np.ascontiguousarray(w, dtype=np.float32)
    halves = L // (NCORES // B)  # 2048
    in_maps = []
    for core in range(NCORES):
        b, h = divmod(core, NCORES // B)
        lo = h * halves
        shard = np.zeros((TOK + PAD, D), dtype=np.float32)
        if lo == 0:
            shard[PAD:] = x[b, lo:lo + TOK]
        else:
            shard[:] = x[b, lo - PAD:lo + TOK]
        in_maps.append({"xs": shard, "wt": w})
    return in_maps


def run(x, w, trace=False, **kw):
    nc = build_nc()
    res = run_bass_kernel_spmd(
        nc, make_in_maps(x, w), core_ids=list(range(NCORES)), trace=trace, **kw
    )
    halves = L // (NCORES // B)
    out = np.empty((B, L, D), dtype=np.float32)
    for core in range(NCORES):
        b, h = divmod(core, NCORES // B)
        out[b, h * halves:(h + 1) * halves] = res.results[core]["y"]
    return out, res


def kernel(x, w):
    out, _ = run(x, w, trace=False)
    return out


# revision 13
# speedup vs baseline: 2.3791x; 1.1168x over previous
"""Causal grouped Conv1d on 8 Trainium2 NeuronCores.

Problem: x [B=4, L=4096, D=2048] f32, w [K=4, D/G=256, D=2048] f32, G=8 groups.
out[b, l, o] = sum_{k, i} x[b, l-3+k, g(o)*256 + i] * w[k, i, o]   (causal pad 3)

Sharding: data-parallel over (B, L/2) -> 8 shards of 2048 tokens each.
Each core gets its token rows plus a 3-row left halo (zeros at batch start).

Per-core kernel (Bass/Tile):
  - PE-transpose x tiles [128 tok, 128 cin] -> xT [128 cin, 131 tok] (incl halo)
  - grouped matmuls: psum[128 tok, 256 och] += xT[:, k:k+128].T @ w[k, cin, och]
    accumulating over k in 0..3 and the group's two 128-cin chunks.
"""

import numpy as np

import concourse.bass as bass
import concourse.mybir as mybir
import concourse.tile as tile
from concourse import bacc, masks
from concourse.bass_utils import run_bass_kernel_spmd

B, L, D, K, G = 4, 4096, 2048, 4, 8
CG = D // G          # 256 channels per group
NCORES = 8
TOK = (B * L) // NCORES   # 2048 tokens per core
TT = 128                  # token tile
NT = TOK // TT            # 16 token tiles
NCHUNK = D // 128         # 16 cin chunks of 128
PAD = K - 1               # 3 (causal left pad)

DT = mybir.dt.float32     # storage/transpose dtype
F32 = mybir.dt.float32
F32R = mybir.dt.float32r
MM_F32R = True            # run conv matmuls in float32r


TB = 512                  # token block for the matmul moving dim
NB = TOK // TB            # 4 token blocks per core


def _emit(tc, nc, xs, wt, y):
    """y is [D, TOK] (transposed); host un-transposes."""
    import contextlib
    ctx = contextlib.ExitStack()
    mmdt = F32R if MM_F32R else DT
    with ctx:
        constp = ctx.enter_context(tc.tile_pool(name="constp", bufs=1))
        wp = ctx.enter_context(tc.tile_pool(name="wp", bufs=1))
        xinp = ctx.enter_context(tc.tile_pool(name="xinp", bufs=6))
        halop = ctx.enter_context(tc.tile_pool(name="halop", bufs=2))
        xtp = ctx.enter_context(tc.tile_pool(name="xtp", bufs=6))
        outp = ctx.enter_context(tc.tile_pool(name="outp", bufs=4))
        pm = ctx.enter_context(tc.tile_pool(name="pm", bufs=2, space="PSUM"))
        ph = ctx.enter_context(tc.tile_pool(name="ph", bufs=2, space="PSUM"))
        po = ctx.enter_context(tc.tile_pool(name="po", bufs=3, space="PSUM"))

        ident = constp.tile([128, 128], DT)
        masks.make_identity(nc, ident[:])

        # Weights resident in SBUF: w_sb[(k, j)] = [128 cin, 2048 och]
        wsb = {}
        for k in range(K):
            for j in range(2):
                w_t = wp.tile([128, D], mmdt, name=f"w_{k}_{j}")
                if MM_F32R:
                    # SWDGE cast fp32 -> fp32r (rounds to fp32r grid)
                    nc.gpsimd.dma_start(w_t[:], wt[k, j * 128:(j + 1) * 128, :])
                else:
                    nc.sync.dma_start(w_t[:], wt[k, j * 128:(j + 1) * 128, :])
                wsb[(k, j)] = w_t

        for t in range(NB):
            t0 = t * TB
            xms = []
            for i in range(TB // TT):
                xm = xinp.tile([TT, D], DT, name="xm")
                nc.sync.dma_start(
                    xm[:], xs[PAD + t0 + i * TT: PAD + t0 + (i + 1) * TT, :]
                )
                xms.append(xm)
            xh = halop.tile([PAD, D], DT, name="xh")
            nc.sync.dma_start(xh[:], xs[t0: t0 + PAD, :])

            def make_xt(c):
                pmt = pm.tile([128, TB], DT, name="pmt")
                for i in range(TB // TT):
                    nc.tensor.transpose(
                        pmt[:, i * TT:(i + 1) * TT],
                        xms[i][:, c * 128:(c + 1) * 128],
                        ident[:],
                    )
                pht = ph.tile([128, PAD], DT, name="pht")
                nc.tensor.transpose(
                    pht[:], xh[:, c * 128:(c + 1) * 128], ident[0:PAD, 0:PAD]
                )
                xt_t = xtp.tile([128, TB + PAD], mmdt, name="xt_t")
                nc.vector.tensor_copy(xt_t[:, PAD:], pmt[:])
                nc.vector.tensor_copy(xt_t[:, 0:PAD], pht[:])
                return xt_t

            for g in range(G):
                xt_pair = [make_xt(2 * g + 0), make_xt(2 * g + 1)]
                for cc in (2 * g, 2 * g + 1):
                    pot = po.tile([128, TB], F32, name="pot")
                    first = True
                    for j in range(2):
                        xt_t = xt_pair[j]
                        for k in range(K):
                            nc.tensor.matmul(
                                pot[:],
                                wsb[(k, j)][:, cc * 128:(cc + 1) * 128],
                                xt_t[:, k:k + TB],
                                start=first,
                                stop=(j == 1 and k == K - 1),
                            )
                            first = False
                    ot = outp.tile([128, TB], F32, name="ot")
                    nc.vector.tensor_copy(ot[:], pot[:])
                    nc.sync.dma_start(
                        y[cc * 128:(cc + 1) * 128, t0:t0 + TB], ot[:]
                    )


_NC_CACHE = None


def build_nc():
    global _NC_CACHE
    if _NC_CACHE is not None:
        return _NC_CACHE
    nc = bacc.Bacc(
        "TRN2", target_bir_lowering=False, debug=False, num_devices=NCORES
    )
    xs = nc.dram_tensor("xs", [TOK + PAD, D], DT, kind="ExternalInput").ap()
    wt = nc.dram_tensor("wt", [K, CG, D], DT, kind="ExternalInput").ap()
    y = nc.dram_tensor("y", [D, TOK], F32, kind="ExternalOutput").ap()
    with tile.TileContext(nc) as tc:
        _emit(tc, nc, xs, wt, y)
    nc.compile()
    _NC_CACHE = nc
    return nc


def make_in_maps(x, w):
    """Shard full x [B, L, D] into 8 per-core inputs with causal halo rows."""
    x = np.ascontiguousarray(x, dtype=np.float32)
    w = np.ascontiguousarray(w, dtype=np.float32)
    halves = L // (NCORES // B)  # 2048
    in_maps = []
    for core in range(NCORES):
        b, h = divmod(core, NCORES // B)
        lo = h * halves
        shard = np.zeros((TOK + PAD, D), dtype=np.float32)
        if lo == 0:
            shard[PAD:] = x[b, lo:lo + TOK]
        else:
            shard[:] = x[b, lo - PAD:lo + TOK]
        in_maps.append({"xs": shard, "wt": w})
    return in_maps


def run(x, w, trace=False, **kw):
    nc = build_nc()
    res = run_bass_kernel_spmd(
        nc, make_in_maps(x, w), core_ids=list(range(NCORES)), trace=trace, **kw
    )
    halves = L // (NCORES // B)
    out = np.empty((B, L, D), dtype=np.float32)
    for core in range(NCORES):
        b, h = divmod(core, NCORES // B)
        out[b, h * halves:(h + 1) * halves] = res.results[core]["y"].T
    return out, res


def kernel(x, w):
    out, _ = run(x, w, trace=False)
    return out


# revision 17
# speedup vs baseline: 2.7836x; 1.1700x over previous
"""Causal grouped Conv1d on 8 Trainium2 NeuronCores.

Problem: x [B=4, L=4096, D=2048] f32, w [K=4, D/G=256, D=2048] f32, G=8 groups.
out[b, l, o] = sum_{k, i} x[b, l-3+k, g(o)*256 + i] * w[k, i, o]   (causal pad 3)

Sharding: data-parallel over (B, L/2) -> 8 shards of 2048 tokens each.
Each core gets its token rows plus a 3-row left halo (zeros at batch start).

Per-core kernel (Bass/Tile):
  - PE-transpose x tiles [128 tok, 128 cin] -> xT [128 cin, 131 tok] (incl halo)
  - grouped matmuls: psum[128 tok, 256 och] += xT[:, k:k+128].T @ w[k, cin, och]
    accumulating over k in 0..3 and the group's two 128-cin chunks.
"""

import numpy as np

import concourse.bass as bass
import concourse.mybir as mybir
import concourse.tile as tile
from concourse import bacc, masks
from concourse.bass_utils import run_bass_kernel_spmd

B, L, D, K, G = 4, 4096, 2048, 4, 8
CG = D // G          # 256 channels per group
NCORES = 8
TOK = (B * L) // NCORES   # 2048 tokens per core
TT = 128                  # token tile
NT = TOK // TT            # 16 token tiles
NCHUNK = D // 128         # 16 cin chunks of 128
PAD = K - 1               # 3 (causal left pad)

DT = mybir.dt.float32     # storage/transpose dtype
F32 = mybir.dt.float32
F32R = mybir.dt.float32r
MM_F32R = True            # run conv matmuls in float32r


TB = 512                  # token block for the matmul moving dim
NB = TOK // TB            # 4 token blocks per core


def _emit(tc, nc, xs, wt, y):
    """y is [D, TOK] (transposed); host un-transposes."""
    import contextlib
    ctx = contextlib.ExitStack()
    mmdt = F32R if MM_F32R else DT
    with ctx:
        constp = ctx.enter_context(tc.tile_pool(name="constp", bufs=1))
        wp = ctx.enter_context(tc.tile_pool(name="wp", bufs=1))
        xinp = ctx.enter_context(tc.tile_pool(name="xinp", bufs=9))
        halop = ctx.enter_context(tc.tile_pool(name="halop", bufs=2))
        xtp = ctx.enter_context(tc.tile_pool(name="xtp", bufs=6))
        outp = ctx.enter_context(tc.tile_pool(name="outp", bufs=4))
        pm = ctx.enter_context(tc.tile_pool(name="pm", bufs=2, space="PSUM"))
        ph = ctx.enter_context(tc.tile_pool(name="ph", bufs=2, space="PSUM"))
        po = ctx.enter_context(tc.tile_pool(name="po", bufs=4, space="PSUM"))

        ident = constp.tile([128, 128], DT)
        masks.make_identity(nc, ident[:])

        # Weights resident in SBUF: w_sb[(k, j)] = [128 cin, 2048 och]
        wsb = {}
        for k in range(K):
            for j in range(2):
                w_t = wp.tile([128, D], mmdt, name=f"w_{k}_{j}")
                nc.sync.dma_start(w_t[:], wt[k, j * 128:(j + 1) * 128, :])
                wsb[(k, j)] = w_t

        for t in range(NB):
            t0 = t * TB
            xms = []
            for i in range(TB // TT):
                xm = xinp.tile([TT, D], DT, name="xm")
                nc.sync.dma_start(
                    xm[:], xs[PAD + t0 + i * TT: PAD + t0 + (i + 1) * TT, :]
                )
                xms.append(xm)
            xh = halop.tile([PAD, D], DT, name="xh")
            nc.sync.dma_start(xh[:], xs[t0: t0 + PAD, :])

            def make_xt(c):
                pmt = pm.tile([128, TB], DT, name="pmt")
                for i in range(TB // TT):
                    nc.tensor.transpose(
                        pmt[:, i * TT:(i + 1) * TT],
                        xms[i][:, c * 128:(c + 1) * 128],
                        ident[:],
                    )
                pht = ph.tile([128, PAD], DT, name="pht")
                nc.tensor.transpose(
                    pht[:], xh[:, c * 128:(c + 1) * 128], ident[0:PAD, 0:PAD]
                )
                xt_t = xtp.tile([128, TB + PAD], mmdt, name="xt_t")
                nc.vector.tensor_copy(xt_t[:, PAD:], pmt[:])
                nc.vector.tensor_copy(xt_t[:, 0:PAD], pht[:])
                return xt_t

            for g in range(G):
                xt_pair = [make_xt(2 * g + 0), make_xt(2 * g + 1)]
                for cc in (2 * g, 2 * g + 1):
                    pot = po.tile([128, TB], F32, name="pot")
                    first = True
                    for j in range(2):
                        xt_t = xt_pair[j]
                        for k in range(K):
                            nc.tensor.matmul(
                                pot[:],
                                wsb[(k, j)][:, cc * 128:(cc + 1) * 128],
                                xt_t[:, k:k + TB],
                                start=first,
                                stop=(j == 1 and k == K - 1),
                            )
                            first = False
                    ot = outp.tile([128, TB], F32, name="ot")
                    nc.vector.tensor_copy(ot[:], pot[:])
                    nc.sync.dma_start(
                        y[cc * 128:(cc + 1) * 128, t0:t0 + TB], ot[:]
                    )


_NC_CACHE = None


def build_nc():
    global _NC_CACHE
    if _NC_CACHE is not None:
        return _NC_CACHE
    nc = bacc.Bacc(
        "TRN2", target_bir_lowering=False, debug=False, num_devices=NCORES
    )
    xs = nc.dram_tensor("xs", [TOK + PAD, D], DT, kind="ExternalInput").ap()
    wt = nc.dram_tensor(
        "wt", [K, CG, D], F32R if MM_F32R else DT, kind="ExternalInput"
    ).ap()
    y = nc.dram_tensor("y", [D, TOK], F32, kind="ExternalOutput").ap()
    with tile.TileContext(nc) as tc:
        _emit(tc, nc, xs, wt, y)
    nc.compile()
    _NC_CACHE = nc
    return nc


def make_in_maps(x, w):
    """Shard full x [B, L, D] into 8 per-core inputs with causal halo rows."""
    x = np.ascontiguousarray(x, dtype=np.float32)
    w = np.ascontiguousarray(w, dtype=np.float32)
    halves = L // (NCORES // B)  # 2048
    in_maps = []
    for core in range(NCORES):
        b, h = divmod(core, NCORES // B)
        lo = h * halves
        shard = np.zeros((TOK + PAD, D), dtype=np.float32)
        if lo == 0:
            shard[PAD:] = x[b, lo:lo + TOK]
        else:
            shard[:] = x[b, lo - PAD:lo + TOK]
        in_maps.append({"xs": shard, "wt": w})
    return in_maps


def run(x, w, trace=False, **kw):
    nc = build_nc()
    res = run_bass_kernel_spmd(
        nc, make_in_maps(x, w), core_ids=list(range(NCORES)), trace=trace, **kw
    )
    halves = L // (NCORES // B)
    out = np.empty((B, L, D), dtype=np.float32)
    for core in range(NCORES):
        b, h = divmod(core, NCORES // B)
        out[b, h * halves:(h + 1) * halves] = res.results[core]["y"].T
    return out, res


def kernel(x, w):
    out, _ = run(x, w, trace=False)
    return out
